# revision 28
# baseline (speedup 1.0000x reference)
"""
Trainium2 Bass kernel for nn_MultiHeadHierarchicalAttention (v2).

Sharding: 8 cores = (batch b in 0..3) x (block-half in 0..1), 16 blocks/core.
The sentence branch is computed redundantly on both cores of a batch (scaled
0.5); the host sums the two per-batch partial outputs.

v2 restructure vs baseline:
  * token scores run as row-tiled CONCURRENT K=64 matmul pairs (head 2hp on
    PE rows 0:63, head 2hp+1 on 64:127) -> half the PE score cycles.
  * exp is split between ACT (table exp) and DVE (Schraudolph int16->bf16
    bitcast exp, max rel err ~3%) with a tunable tile quota.
  * the per-block sentence/denominator factor is applied batched: S3 writes
    65-wide runs (64 ctx cols + ones-column denom) at stride 128 into a
    4-bank PSUM tile; DVE does recip + factor + one broadcast TT-mul per
    2-block group into a bf16 stash [hq, dv, gn], then ONE reduce per
    head-pair produces ctx directly. Replaces 512 tiny STT ops.
  * b_vw is folded into vw_sb during the projection copy (TT-add with a
    broadcast bias tile), removing the per-head bias pass.
"""

import sys

sys.path.insert(0, "/opt/trn_rl_repo")

import numpy as np
import ml_dtypes
import concourse.bass as bass
import concourse.tile as tile
from concourse import mybir
from concourse.bass_utils import run_bass_kernel_spmd
from concourse.vector_clock import ScopedClock
from concourse.masks import make_identity

# ---------------------------------------------------------------- constants
B, LQ, NB, NT = 4, 512, 32, 256
D, H, DK, DV = 512, 8, 64, 64
NBH = NB // 2  # blocks per core
NTOK = NBH * NT  # tokens per core = 4096
SCALE = 0.125
FP = mybir.dt.float32
FR = mybir.dt.float32r
BF = mybir.dt.bfloat16
I16 = mybir.dt.int16
N_CORES = 8

AX = mybir.AxisListType.X
ALU = mybir.AluOpType
ACTF = mybir.ActivationFunctionType

# Schraudolph bf16 exp constants: int16(x*A + B) bitcast to bf16 ~ exp(x)
SCH_A = float((2.0**7) / np.log(2.0))
SCH_B = float(16256 - 5.5)
# how many of the 128 exp tiles run on DVE instead of ACT (load balance)
N_DVE_EXP = 24
DVE_KS = {8 + int(round(i * 119.0 / (N_DVE_EXP - 1))) for i in range(N_DVE_EXP)}
FP16 = mybir.dt.float16
F8 = mybir.dt.float8e4
KW_UP = 64.0  # host scales W_kw/W_vw by 64 so fp8 weights aren't subnormal


# --------------------------------------------------------- drain workaround
def _patched_drain_and_barrier(self, tick_clock, wait_clock):
    nop_inst = self.nc.sync.nop(nofuse=True)
    wait_clock.add_sem_waits(nop_inst.ins, ScopedClock({None: tick_clock.global_clock}))
    waits = list(nop_inst.ins.sync_info.on_wait or [])
    if len(waits) > 1:
        nop_inst.ins.sync_info.on_wait = waits[:1]
        rest = waits[1:]
        while rest:
            extra = self.nc.sync.nop(nofuse=True)
            if extra.ins.sync_info is None:
                extra.ins.sync_info = mybir.SyncInfo(on_wait=[], on_update=[])
            extra.ins.sync_info.on_wait = rest[:1]
            rest = rest[1:]
    self.nc.sync.drain()
    self.nc.all_engine_barrier()
    assert self.sems is not None
    popped = self.nc._tile_sem_poison_stack.pop()
    assert popped is self._sem_poison
    self.nc.clear_and_free_semaphores(list(self.sems.allocated().values()))
    self.nc.all_engine_barrier()


tile.TileContext._drain_and_barrier = _patched_drain_and_barrier


def _r(ap):
    return ap.bitcast(mybir.dt.float32r)


_NO_SPLIT_OPCODES = {
    "CollectiveCompute",
    "EventSemaphore",
}
_split_counter = [0]


def _split_multi_waits(nc):
    n_split = 0
    for fn in nc.m.functions:
        for bb in fn.blocks:
            changed = False
            out = []
            for inst in bb.instructions:
                si = inst.sync_info
                if (
                    si is not None
                    and si.on_wait
                    and len(list(si.on_wait)) > 1
                    and inst.opcode not in _NO_SPLIT_OPCODES
                ):
                    waits = list(si.on_wait)
                    for w in waits[:-1]:
                        _split_counter[0] += 1
                        nop = mybir.InstNoOp(name=f"I-wsplit-{_split_counter[0]}")
                        nop.engine = inst.engine
                        nop.sync_info = mybir.SyncInfo(on_wait=[w], on_update=[])
                        out.append(nop)
                        n_split += 1
                    si.on_wait = waits[-1:]
                    changed = True
                out.append(inst)
            if changed:
                bb.instructions = out
    return n_split


def _flat2(ap):
    return ap.rearrange("p a b -> p (a b)")


def _ax(ap, axes):
    """Build an AP on the same tensor with explicit [stride, num] free axes."""
    return bass.AP(ap.tensor, ap.offset, [list(ap.ap[0])] + [list(a) for a in axes])



def _dram_chunks(dram_ap, nk, ncols, col_off=0, row_w=None):
    """AP reading nk row-chunks of 128 from a [nk*128, W] dram tensor as
    [128 part, nk, ncols]."""
    W = row_w if row_w is not None else dram_ap.ap[-1][1]
    return bass.AP(
        dram_ap.tensor,
        dram_ap.offset + col_off,
        [[W, 128], [128 * W, nk], [1, ncols]],
    )


# ------------------------------------------------------------ program build
def build_program():
    nc = bass.Bass("TRN2", target_bir_lowering=False, debug=False, num_devices=N_CORES)

    dt_in = {}
    for name, shape in [
        ("kwT", [D, NTOK]),
        ("vwT", [D, NTOK]),
        ("Wkw", [D, H * DK]),
        ("Wvw", [D, H * DV]),
    ]:
        dt_in[name] = nc.dram_tensor(name, shape, BF, kind="ExternalInput").ap()
    for name, shape in [
        ("qT", [D, LQ]),
        ("ksT", [D, NB]),
        ("vsT", [D, NB]),
        ("Wqs", [D, H * DK]),
        ("Wks", [D, H * DK]),
        ("Wvs", [D, H * DV]),
        ("Wqw", [D, H * DK]),
        ("Wfc", [D + H * DV, D]),
        ("Wfc1", [H * DV, D]),
    ]:
        dt_in[name] = nc.dram_tensor(name, shape, FR, kind="ExternalInput").ap()
    for name, shape in [
        ("bqsT", [128, 4]),
        ("bksT", [128, 4]),
        ("bqwT", [128, 4]),
        ("bkwT", [128, 4]),
        ("bvsT", [128, 4]),
        ("bfc1T", [128, 4]),
        ("bfcT", [128, 4]),
        ("bvw", [H * DV]),
    ]:
        dt_in[name] = nc.dram_tensor(name, shape, FP, kind="ExternalInput").ap()
    outT_d = nc.dram_tensor("outT", [D, LQ], FP, kind="ExternalOutput").ap()

    with tile.TileContext(nc) as tc:
        # ------------------------------------------------ persistent pools
        ppool_cm = tc.tile_pool(name="persist", bufs=1)
        ppool = ppool_cm.__enter__()
        scpool_cm = tc.tile_pool(name="scps", bufs=2, space="PSUM")
        scpool = scpool_cm.__enter__()
        vpool_cm = tc.tile_pool(name="valps", bufs=2, space="PSUM")
        vpool = vpool_cm.__enter__()
        ewpool_cm = tc.tile_pool(name="ewp", bufs=6)
        ewpool = ewpool_cm.__enter__()
        smpool_cm = tc.tile_pool(name="small", bufs=6)
        smpool = smpool_cm.__enter__()

        ident = ppool.tile([128, 128], FP, tag="ident")
        ident16 = ppool.tile([128, 128], FP16, tag="ident16")

        # persistent sbuf tensors
        qw_cmp = ppool.tile([128, 4, LQ], BF, tag="qw_cmp")
        ks_sb = ppool.tile([128, 4, NB], FR, tag="ks_sb")
        attn_sb = ppool.tile([128, 4, H, NB], FP, tag="attn_sb")
        fc1T_sb = ppool.tile([128, 4, LQ], FR, tag="fc1T")
        kw_sb = ppool.tile([128, 4, NTOK], BF, tag="kw_sb")
        vw_sb = ppool.tile([128, NTOK // 128, H, DV + 1], BF, tag="vw_sb")
        vs_sb = ppool.tile([NB, H * DV], FR, tag="vs_sb")
        Wkw_sb = ppool.tile([128, 4, H * DK], BF, tag="Wkw")
        Wvw_sb = ppool.tile([128, 4, H * DV], BF, tag="Wvw")
        ctx_sT = ppool.tile([128, 4, LQ], FR, tag="ctx_sT")

        stpool_cm = tc.tile_pool(name="stage", bufs=3)
        stpool = stpool_cm.__enter__()

        ctx_wT = ppool.tile([128, 4, LQ], FR, tag="ctx_wT")

        # ones column of vw_sb (denominator trick)
        nc.vector.memset(vw_sb[:, :, :, DV : DV + 1], 1.0)

        stg_cur = {}

        def kw_piece(sg, mo, j):
            """4 accumulating MMs -> [128,512] psum -> ACT copy to kw_sb."""
            if mo == 0 and j == 0:
                stg = stpool.tile([128, 4, 1024], BF, tag="stg", name="kwstg")
                nc.sync.dma_start(
                    out=stg[:], in_=_dram_chunks(dt_in["kwT"], 4, 1024, col_off=sg * 1024)
                )
                stg_cur["kw"] = stg
            stg = stg_cur["kw"]
            ps = scpool.tile([128, 2, 512], FP, tag="sc", name="kwps")
            for k in range(4):
                nc.tensor.matmul(
                    ps[:, 0, :],
                    Wkw_sb[:, k, mo * 128 : (mo + 1) * 128],
                    stg[:, k, j * 512 : (j + 1) * 512],
                    start=(k == 0),
                    stop=(k == 3),
                )
            # b_kw cancels in softmax/ratio; pure copy on ACT
            nc.scalar.activation(
                kw_sb[:, mo, sg * 1024 + j * 512 : sg * 1024 + (j + 1) * 512],
                ps[:, 0, :],
                ACTF.Copy,
            )

        def vw_piece(sg, tp, j):
            """One token-chunk (128 tok) of vw: 4 MMs + DVE bias-fold copy."""
            if tp == 0 and j == 0:
                stg = stpool.tile([128, 4, 1024], BF, tag="stg", name="vwstg")
                nc.sync.dma_start(
                    out=stg[:], in_=_dram_chunks(dt_in["vwT"], 4, 1024, col_off=sg * 1024)
                )
                stg_cur["vw"] = stg
            stg = stg_cur["vw"]
            tcl = tp * 2 + j
            ps = scpool.tile([128, 2, 512], FP, tag="sc", name="vwps")
            for k in range(4):
                nc.tensor.matmul(
                    ps[:, 0, :],
                    stg[:, k, tcl * 128 : (tcl + 1) * 128],
                    Wvw_sb[:, k, :],
                    start=(k == 0),
                    stop=(k == 3),
                )
            # vw copy with b_vw folded in: out[tok, (h,dv)] = ps + bvw
            tc0 = sg * 8 + tcl
            out_ap = _ax(vw_sb[:, tc0, 0, 0], [[DV + 1, H], [1, DV]])
            in0 = ps[:, 0, :].rearrange("p (h x) -> p h x", x=DV)
            in1 = _ax(bvw_bc, [[DV, H], [1, DV]])
            nc.vector.scalar_tensor_tensor(
                out_ap, in0, 1.0, in1, op0=ALU.mult, op1=ALU.add
            )

        PIECES = []
        for sg in range(4):
            for mo in range(4):
                for j in range(2):
                    PIECES.append((sg, "kw", mo, j))
            for tp in range(4):
                for j in range(2):
                    PIECES.append((sg, "vw", tp, j))

        def emit_piece(p):
            sg, kind, a, j = p
            if kind == "kw":
                kw_piece(sg, a, j)
            else:
                vw_piece(sg, a, j)

        # ------------------------------------------- phase 1+2: small branch
        with tc.tile_pool(name="phaseA", bufs=1) as apool, tc.tile_pool(
            name="wring", bufs=2
        ) as wpool:
            qT_sb = apool.tile([128, 4, LQ], FR, tag="qT_sb")
            ksT_sb = apool.tile([128, 4, NB], FR, tag="ksT_sb")
            vsT_sb = apool.tile([128, 4, NB], FR, tag="vsT_sb")
            qs_sb = apool.tile([128, 4, LQ], FR, tag="qs_sb")

            def load_W(wn, wdt=FR):
                t = wpool.tile([128, 4, D], wdt, tag="Wring", name=wn)
                nc.sync.dma_start(out=t[:], in_=_dram_chunks(dt_in[wn], 4, D))
                return t

            # critical-path DMAs first: qT + b_qw + Wqw feed the first matmuls
            nc.sync.dma_start(out=qT_sb[:], in_=_dram_chunks(dt_in["qT"], 4, LQ))
            btiles = {}
            for bn in ["bqwT"]:
                t = ppool.tile([128, 4], FP, tag=bn)
                nc.sync.dma_start(out=t[:], in_=dt_in[bn][:])
                btiles[bn] = t
            make_identity(nc, ident)
            make_identity(nc, ident16)

            # qw projection -> compact [128, hp, LQ] bf16 (head 2hp on 0:64,
            # head 2hp+1 on 64:128 partitions)
            Wqw_t = load_W("Wqw")
            for mo in range(4):
                ps = scpool.tile([128, 2, 512], FP, tag="sc", name="qwps")
                for k in range(4):
                    nc.tensor.matmul(
                        ps[:, 0, :],
                        Wqw_t[:, k, mo * 128 : (mo + 1) * 128],
                        qT_sb[:, k, :],
                        start=(k == 0),
                        stop=(k == 3),
                    )
                nc.vector.tensor_scalar_add(
                    qw_cmp[:, mo, :], ps[:, 0, :], btiles["bqwT"][:, mo : mo + 1]
                )
            nc.sync.dma_start(out=Wkw_sb[:], in_=_dram_chunks(dt_in["Wkw"], 4, H * DK))
            nc.sync.dma_start(out=Wvw_sb[:], in_=_dram_chunks(dt_in["Wvw"], 4, H * DV))
            # remaining phase-A DMAs
            nc.sync.dma_start(out=ksT_sb[:], in_=_dram_chunks(dt_in["ksT"], 4, NB))
            nc.sync.dma_start(out=vsT_sb[:], in_=_dram_chunks(dt_in["vsT"], 4, NB))
            for bn in ["bqsT", "bvsT", "bfc1T", "bfcT"]:
                t = ppool.tile([128, 4], FP, tag=bn)
                nc.sync.dma_start(out=t[:], in_=dt_in[bn][:])
                btiles[bn] = t
            bvw_bc = ppool.tile([128, H * DV], FP, tag="bvw_bc")
            src = dt_in["bvw"]
            bcast_ap = bass.AP(src.tensor, src.offset, [[0, 128]] + [list(x) for x in src.ap])
            nc.sync.dma_start(out=bvw_bc[:], in_=bcast_ap)
            for p in PIECES[0:16]:
                emit_piece(p)
            # qs projection
            Wqs_t = load_W("Wqs")
            for mo in range(4):
                ps = scpool.tile([128, 2, 512], FP, tag="sc", name="qsps")
                for k in range(4):
                    nc.tensor.matmul(
                        ps[:, 0, :],
                        Wqs_t[:, k, mo * 128 : (mo + 1) * 128],
                        qT_sb[:, k, :],
                        start=(k == 0),
                        stop=(k == 3),
                    )
                nc.vector.tensor_scalar_add(
                    qs_sb[:, mo, :], ps[:, 0, :], btiles["bqsT"][:, mo : mo + 1]
                )

            # ks projection (b_ks cancels in sentence softmax)
            Wks_t = load_W("Wks")
            for mo in range(4):
                ps = scpool.tile([128, 2, 512], FP, tag="sc", name="ksps")
                for k in range(4):
                    nc.tensor.matmul(
                        ps[:, 0, 0:NB],
                        Wks_t[:, k, mo * 128 : (mo + 1) * 128],
                        ksT_sb[:, k, :],
                        start=(k == 0),
                        stop=(k == 3),
                    )
                nc.scalar.activation(ks_sb[:, mo, :], ps[:, 0, 0:NB], ACTF.Copy)

            # vs projection: out [nb, hdv]
            Wvs_t = load_W("Wvs")
            ps = scpool.tile([128, 2, 512], FP, tag="sc", name="vsps")
            for k in range(4):
                nc.tensor.matmul(
                    ps[0:NB, 0, :],
                    vsT_sb[:, k, :],
                    Wvs_t[:, k, :],
                    start=(k == 0),
                    stop=(k == 3),
                )
            nc.scalar.activation(vs_sb[:, :], ps[0:NB, 0, :], ACTF.Copy)

            # sentence attention: scores + exp per head, then batched softmax
            ews_all = apool.tile([128, 4, H, NB], FP, tag="ews_all")
            for h in range(H):
                hp, po = h // 2, (h % 2) * 64
                ps = scpool.tile([128, 2, 512], FP, tag="sc", name="sattps")
                for qo in range(4):
                    nc.tensor.matmul(
                        ps[:, 0, qo * NB : (qo + 1) * NB],
                        qs_sb[po : po + 64, hp, qo * 128 : (qo + 1) * 128],
                        ks_sb[po : po + 64, hp, :],
                        start=True,
                        stop=True,
                    )
                nc.scalar.activation(
                    ews_all[:, :, h, :],
                    ps[:, 0, 0 : 4 * NB].rearrange("p (a x) -> p a x", x=NB),
                    ACTF.Exp,
                    scale=SCALE,
                )
            # batched softmax over nb: den[128, (qo,h)] -> recip -> mul
            den_t = apool.tile([128, 4, H], FP, tag="den_t")
            nc.vector.tensor_reduce(den_t[:], ews_all[:], AX, ALU.add)
            rec_t = apool.tile([128, 4, H], FP, tag="rec_t")
            nc.vector.reciprocal(rec_t[:], den_t[:])
            rec_bc = _ax(rec_t, [[H, 4], [1, H], [0, NB]])
            nc.vector.tensor_tensor(attn_sb[:], ews_all[:], rec_bc, op=ALU.mult)

        # ---------------- main loop: projections + token attention pipeline

        # per-iteration state for the 2-deep software pipeline
        ew_store = {}

        t1pool_cm = tc.tile_pool(name="tail1", bufs=1)
        t1pool = t1pool_cm.__enter__()
        aspool_cm = tc.tile_pool(name="asTring", bufs=2)
        aspool = aspool_cm.__enter__()
        accpool_cm = tc.tile_pool(name="accp", bufs=4)
        accpool = accpool_cm.__enter__()
        tmppool_cm = tc.tile_pool(name="tmpp", bufs=4)
        tmppool = tmppool_cm.__enter__()
        Wfc1_sb = t1pool.tile([128, 4, D], FR, tag="Wfc1l")
        nc.sync.dma_start(out=Wfc1_sb[:], in_=_dram_chunks(dt_in["Wfc1"], 4, D))

        def sentence_tail(h):
            hp, po = h // 2, (h % 2) * 64
            asT_h = aspool.tile([NB, 4, 128], FR, tag="asT", name="asT_h")
            psT = scpool.tile([128, 2, 512], FP, tag="sc", name="sattT")
            for qo in range(4):
                nc.tensor.transpose(
                    psT[0:NB, 0, qo * 128 : (qo + 1) * 128], attn_sb[:, qo, h, :], ident[:]
                )
            nc.scalar.activation(asT_h[:, :, :], psT[0:NB, 0, :], ACTF.Copy)
            ps = scpool.tile([128, 2, 512], FP, tag="sc", name="ctxsps")
            nc.tensor.matmul(
                ps[0:64, 0, :],
                vs_sb[:, h * 64 : (h + 1) * 64],
                asT_h[:, :, :],
                start=True,
                stop=True,
            )
            nc.vector.tensor_scalar_add(
                ctx_sT[po : po + 64, hp, :],
                ps[0:64, 0, :],
                btiles["bvsT"][po : po + 64, hp : hp + 1],
            )

        def emit_fc1():
            for mo in range(4):
                ps = scpool.tile([128, 2, 512], FP, tag="sc", name="fc1ps")
                for k in range(4):
                    nc.tensor.matmul(
                        ps[:, 0, :],
                        Wfc1_sb[:, k, mo * 128 : (mo + 1) * 128],
                        ctx_sT[:, k, :],
                        start=(k == 0),
                        stop=(k == 3),
                    )
                nc.vector.tensor_scalar(
                    fc1T_sb[:, mo, :],
                    ps[:, 0, :],
                    0.5,
                    btiles["bfc1T"][:, mo : mo + 1],
                    op0=ALU.mult,
                    op1=ALU.add,
                )

        acc_cur = [None] * 4

        def emit_scores(it):
            n, hp = divmod(it, 4)
            # fp32r work first so the bf16 scores+S3 cluster keeps FWL alive
            if it in (5, 7, 9, 11, 13, 15, 17, 19):
                sentence_tail((it - 5) // 2)
            elif it == 21:
                emit_fc1()
            sg, k = it // 16 + 1, it % 16
            if sg <= 3:
                emit_piece(PIECES[sg * 16 + k])
            psA = scpool.tile([128, 2, 512], FP, tag="sc", name="psA")
            psB = scpool.tile([128, 2, 512], FP, tag="sc", name="psB")
            for tl in range(2):
                tcg = 2 * n + tl
                nc.tensor.matmul(
                    psA[:, tl, :],
                    kw_sb[0:64, hp, tcg * 128 : (tcg + 1) * 128],
                    qw_cmp[0:64, hp, :],
                    start=True,
                    stop=True,
                    tile_position=(0, 0),
                )
                nc.tensor.matmul(
                    psB[:, tl, :],
                    kw_sb[64:128, hp, tcg * 128 : (tcg + 1) * 128],
                    qw_cmp[64:128, hp, :],
                    start=True,
                    stop=True,
                    tile_position=(64, 0),
                )
            tiles = []
            for j, psx in enumerate((psA, psB)):
                ew_t = ewpool.tile([128, 2, 512], BF, tag="ew", name="ew_t")
                if (2 * it + j) in DVE_KS:
                    nc.vector.tensor_scalar(
                        ew_t.bitcast(I16)[:],
                        psx[:],
                        SCH_A * SCALE,
                        SCH_B,
                        op0=ALU.mult,
                        op1=ALU.add,
                    )
                else:
                    nc.scalar.activation(_flat2(ew_t), _flat2(psx), ACTF.Exp, scale=SCALE)
                tiles.append(ew_t)
            ew_store[it] = tiles

        def emit_s3(it):
            n, hp = divmod(it, 4)
            tiles = ew_store.pop(it)
            # per-iteration vals tile: 8 runs of 65 at stride 128 (2 banks);
            # col 64 = denominator (ones column of vw)
            vals = vpool.tile([128, 8, 128], FP, tag="vals", name="vals")
            for hl in range(2):
                h = 2 * hp + hl
                ew_t = tiles[hl]
                for qo in range(4):
                    hq = hl * 4 + qo
                    for tl in range(2):
                        tcg = 2 * n + tl
                        nc.tensor.matmul(
                            vals[:, hq, 0 : DV + 1],
                            ew_t[:, tl, qo * 128 : (qo + 1) * 128],
                            vw_sb[:, tcg, h, :],
                            start=(tl == 0),
                            stop=(tl == 1),
                        )
            if n == 0:
                acc_cur[hp] = accpool.tile([128, 4, 2, DV], FP16, tag="acc", name="acc")
            acc = acc_cur[hp]
            # fac[hl,qo] = attn_s * (1/den)
            rec = smpool.tile([128, 2, 4], FP, tag="rec")
            den_ap = _ax(vals[:, 0, DV], [[512, 2], [128, 4]])
            nc.vector.reciprocal(rec[:], den_ap)
            fac = smpool.tile([128, 2, 4], FP, tag="fac")
            attn_ap = _ax(attn_sb[:, 0, 2 * hp, n], [[NB, 2], [H * NB, 4]])
            nc.vector.tensor_tensor(fac[:], rec[:], attn_ap, op=ALU.mult)
            # acc (+)= vals * fac ; iterate (hl, qo, dv), acc laid out (qo, hl, dv)
            in0 = _ax(vals[:, 0, 0], [[512, 2], [128, 4], [1, DV]])
            fac_bc = _ax(fac[:, 0, 0], [[4, 2], [1, 4], [0, DV]])
            acc_w = _ax(acc[:, 0, 0, 0], [[DV, 2], [2 * DV, 4], [1, DV]])
            if n == 0:
                nc.vector.tensor_tensor(acc_w, in0, fac_bc, op=ALU.mult)
            else:
                tmp = tmppool.tile([128, 4, 2, DV], FP16, tag="tmp", name="tmp")
                tmp_w = _ax(tmp[:, 0, 0, 0], [[DV, 2], [2 * DV, 4], [1, DV]])
                nc.vector.tensor_tensor(tmp_w, in0, fac_bc, op=ALU.mult)
                nc.vector.tensor_tensor(acc[:], tmp[:], acc[:], op=ALU.add)
            if n == 15:
                # transpose ctx_w pair -> ctx_wT[:, hp, :]
                ps = scpool.tile([128, 2, 512], FP, tag="sc", name="ctps")
                psh = ps.bitcast(FP16)  # [128, 2, 1024] fp16 view
                for qo in range(4):
                    nc.tensor.transpose(
                        psh[:, 0, qo * 128 : (qo + 1) * 128],
                        acc[:, qo, :, :].rearrange("p a b -> p (a b)"),
                        ident16[:],
                    )
                nc.scalar.activation(ctx_wT[:, hp, :], psh[:, 0, 0:512], ACTF.Copy)

        for it in range(64 + 2):
            if it < 64:
                emit_scores(it)
            if it >= 2:
                emit_s3(it - 2)

        tmppool_cm.__exit__(None, None, None)
        accpool_cm.__exit__(None, None, None)
        aspool_cm.__exit__(None, None, None)
        t1pool_cm.__exit__(None, None, None)
        stpool_cm.__exit__(None, None, None)

        # ---------------- final fc
        lpool_cm = tc.tile_pool(name="late", bufs=1)
        lpool = lpool_cm.__enter__()
        outT_sb = lpool.tile([128, 4, LQ], FP, tag="outT_sb")
        Wfc_sb = lpool.tile([128, 8, D], FR, tag="Wfc")
        nc.sync.dma_start(out=Wfc_sb[:], in_=_dram_chunks(dt_in["Wfc"], 8, D))
        for mo in range(4):
            ps = scpool.tile([128, 2, 512], FP, tag="sc", name="fcps")
            for cc in range(4):
                nc.tensor.matmul(
                    ps[:, 0, :],
                    Wfc_sb[:, cc, mo * 128 : (mo + 1) * 128],
                    fc1T_sb[:, cc, :],
                    start=(cc == 0),
                    stop=False,
                )
            for cc in range(4):
                nc.tensor.matmul(
                    ps[:, 0, :],
                    Wfc_sb[:, 4 + cc, mo * 128 : (mo + 1) * 128],
                    ctx_wT[:, cc, :],
                    start=False,
                    stop=(cc == 3),
                )
            nc.vector.tensor_scalar_add(
                outT_sb[:, mo, :], ps[:, 0, :], btiles["bfcT"][:, mo : mo + 1]
            )
            nc.sync.dma_start(out=outT_d[mo * 128 : (mo + 1) * 128, :], in_=outT_sb[:, mo, :])

        lpool_cm.__exit__(None, None, None)
        smpool_cm.__exit__(None, None, None)
        ewpool_cm.__exit__(None, None, None)
        vpool_cm.__exit__(None, None, None)
        scpool_cm.__exit__(None, None, None)
        ppool_cm.__exit__(None, None, None)

    ns = _split_multi_waits(nc)
    print(f"[kernel] split {ns} extra sem waits onto NOPs", file=sys.stderr)
    return nc


_NC_CACHE = None


def _get_nc():
    global _NC_CACHE
    if _NC_CACHE is None:
        _NC_CACHE = build_program()
    return _NC_CACHE


def make_in_maps(inputs):
    f = lambda x: np.ascontiguousarray(np.asarray(x, dtype=np.float32))
    q, k_w, v_w, k_s, v_s = (f(inputs[n]) for n in ["q", "k_w", "v_w", "k_s", "v_s"])
    W = {n: f(inputs[n]) for n in inputs if n.startswith(("W_", "b_"))}

    def bT(v, scale=1.0):
        return np.ascontiguousarray((v * scale).reshape(4, 128).T)

    bf = ml_dtypes.bfloat16
    shared = {
        "Wqs": W["W_qs"], "Wks": W["W_ks"], "Wvs": W["W_vs"],
        "Wqw": W["W_qw"], "Wkw": W["W_kw"].astype(bf), "Wvw": W["W_vw"].astype(bf),
        "Wfc1": W["W_fc1"], "Wfc": W["W_fc"],
        "bqsT": bT(W["b_qs"]), "bksT": bT(W["b_ks"]), "bqwT": bT(W["b_qw"]),
        "bkwT": bT(W["b_kw"]), "bvsT": bT(W["b_vs"]),
        "bfc1T": bT(W["b_fc1"], 0.5), "bfcT": bT(W["b_fc"], 0.5),
        "bvw": W["b_vw"],
    }
    in_maps = []
    for c in range(N_CORES):
        b, half = divmod(c, 2)
        blk = slice(half * NBH, half * NBH + NBH)
        ks_r = np.roll(k_s[b], -half * NBH, axis=0)
        vs_r = np.roll(v_s[b], -half * NBH, axis=0)
        m = dict(shared)
        m["qT"] = np.ascontiguousarray(q[b].T)
        m["kwT"] = np.ascontiguousarray(k_w[b, blk].reshape(NTOK, D).T.astype(bf))
        m["vwT"] = np.ascontiguousarray(v_w[b, blk].reshape(NTOK, D).T.astype(bf))
        m["ksT"] = np.ascontiguousarray(ks_r.T)
        m["vsT"] = np.ascontiguousarray(vs_r.T)
        in_maps.append(m)
    return in_maps


def run_cores(inputs, trace=False):
    nc = _get_nc()
    in_maps = make_in_maps(inputs)
    res = run_bass_kernel_spmd(nc, in_maps, list(range(N_CORES)), trace=trace)
    return res


def assemble(res):
    out = np.empty((B, LQ, D), dtype=np.float32)
    for b in range(B):
        out[b] = (res.results[2 * b]["outT"] + res.results[2 * b + 1]["outT"]).T
    return out


def kernel(**inputs) -> np.ndarray:
    res = run_cores(inputs, trace=False)
    return assemble(res)


if __name__ == "__main__":
    import reference

    inp = {k: np.asarray(v) for k, v in reference.setup_inputs().items()}
    out = kernel(**inp)
    exp = np.asarray(reference.reference(**inp))
    err = np.abs(out - exp).max() / np.abs(exp).max()
    print("max rel err:", err)


# revision 30
# speedup vs baseline: 1.0101x; 1.0101x over previous
"""
Trainium2 Bass kernel for nn_MultiHeadHierarchicalAttention (v2).

Sharding: 8 cores = (batch b in 0..3) x (block-half in 0..1), 16 blocks/core.
The sentence branch is computed redundantly on both cores of a batch (scaled
0.5); the host sums the two per-batch partial outputs.

v2 restructure vs baseline:
  * token scores run as row-tiled CONCURRENT K=64 matmul pairs (head 2hp on
    PE rows 0:63, head 2hp+1 on 64:127) -> half the PE score cycles.
  * exp is split between ACT (table exp) and DVE (Schraudolph int16->bf16
    bitcast exp, max rel err ~3%) with a tunable tile quota.
  * the per-block sentence/denominator factor is applied batched: S3 writes
    65-wide runs (64 ctx cols + ones-column denom) at stride 128 into a
    4-bank PSUM tile; DVE does recip + factor + one broadcast TT-mul per
    2-block group into a bf16 stash [hq, dv, gn], then ONE reduce per
    head-pair produces ctx directly. Replaces 512 tiny STT ops.
  * b_vw is folded into vw_sb during the projection copy (TT-add with a
    broadcast bias tile), removing the per-head bias pass.
"""

import sys

sys.path.insert(0, "/opt/trn_rl_repo")

import numpy as np
import ml_dtypes
import concourse.bass as bass
import concourse.tile as tile
from concourse import mybir
from concourse.bass_utils import run_bass_kernel_spmd
from concourse.vector_clock import ScopedClock
from concourse.masks import make_identity

# ---------------------------------------------------------------- constants
B, LQ, NB, NT = 4, 512, 32, 256
D, H, DK, DV = 512, 8, 64, 64
NBH = NB // 2  # blocks per core
NTOK = NBH * NT  # tokens per core = 4096
SCALE = 0.125
FP = mybir.dt.float32
FR = mybir.dt.float32r
BF = mybir.dt.bfloat16
I16 = mybir.dt.int16
N_CORES = 8

AX = mybir.AxisListType.X
ALU = mybir.AluOpType
ACTF = mybir.ActivationFunctionType

# Schraudolph bf16 exp constants: int16(x*A + B) bitcast to bf16 ~ exp(x)
SCH_A = float((2.0**7) / np.log(2.0))
SCH_B = float(16256 - 5.5)
# how many of the 128 exp tiles run on DVE instead of ACT (load balance)
N_DVE_EXP = 24
DVE_KS = {8 + int(round(i * 119.0 / (N_DVE_EXP - 1))) for i in range(N_DVE_EXP)}
FP16 = mybir.dt.float16
F8 = mybir.dt.float8e4
KW_UP = 64.0  # host scales W_kw/W_vw by 64 so fp8 weights aren't subnormal


# --------------------------------------------------------- drain workaround
def _patched_drain_and_barrier(self, tick_clock, wait_clock):
    nop_inst = self.nc.sync.nop(nofuse=True)
    wait_clock.add_sem_waits(nop_inst.ins, ScopedClock({None: tick_clock.global_clock}))
    waits = list(nop_inst.ins.sync_info.on_wait or [])
    if len(waits) > 1:
        nop_inst.ins.sync_info.on_wait = waits[:1]
        rest = waits[1:]
        while rest:
            extra = self.nc.sync.nop(nofuse=True)
            if extra.ins.sync_info is None:
                extra.ins.sync_info = mybir.SyncInfo(on_wait=[], on_update=[])
            extra.ins.sync_info.on_wait = rest[:1]
            rest = rest[1:]
    self.nc.sync.drain()
    self.nc.all_engine_barrier()
    assert self.sems is not None
    popped = self.nc._tile_sem_poison_stack.pop()
    assert popped is self._sem_poison
    self.nc.clear_and_free_semaphores(list(self.sems.allocated().values()))
    self.nc.all_engine_barrier()


tile.TileContext._drain_and_barrier = _patched_drain_and_barrier


def _r(ap):
    return ap.bitcast(mybir.dt.float32r)


_NO_SPLIT_OPCODES = {
    "CollectiveCompute",
    "EventSemaphore",
}
_split_counter = [0]


def _split_multi_waits(nc):
    n_split = 0
    for fn in nc.m.functions:
        for bb in fn.blocks:
            changed = False
            out = []
            for inst in bb.instructions:
                si = inst.sync_info
                if (
                    si is not None
                    and si.on_wait
                    and len(list(si.on_wait)) > 1
                    and inst.opcode not in _NO_SPLIT_OPCODES
                ):
                    waits = list(si.on_wait)
                    for w in waits[:-1]:
                        _split_counter[0] += 1
                        nop = mybir.InstNoOp(name=f"I-wsplit-{_split_counter[0]}")
                        nop.engine = inst.engine
                        nop.sync_info = mybir.SyncInfo(on_wait=[w], on_update=[])
                        out.append(nop)
                        n_split += 1
                    si.on_wait = waits[-1:]
                    changed = True
                out.append(inst)
            if changed:
                bb.instructions = out
    return n_split


def _flat2(ap):
    return ap.rearrange("p a b -> p (a b)")


def _ax(ap, axes):
    """Build an AP on the same tensor with explicit [stride, num] free axes."""
    return bass.AP(ap.tensor, ap.offset, [list(ap.ap[0])] + [list(a) for a in axes])



def _dram_chunks(dram_ap, nk, ncols, col_off=0, row_w=None):
    """AP reading nk row-chunks of 128 from a [nk*128, W] dram tensor as
    [128 part, nk, ncols]."""
    W = row_w if row_w is not None else dram_ap.ap[-1][1]
    return bass.AP(
        dram_ap.tensor,
        dram_ap.offset + col_off,
        [[W, 128], [128 * W, nk], [1, ncols]],
    )


# ------------------------------------------------------------ program build
def build_program():
    nc = bass.Bass("TRN2", target_bir_lowering=False, debug=False, num_devices=N_CORES)

    dt_in = {}
    for name, shape in [
        ("kwT", [D, NTOK]),
        ("vwT", [D, NTOK]),
        ("Wkw", [D, H * DK]),
        ("Wvw", [D, H * DV]),
    ]:
        dt_in[name] = nc.dram_tensor(name, shape, BF, kind="ExternalInput").ap()
    for name, shape in [
        ("qT", [D, LQ]),
        ("ksT", [D, NB]),
        ("vsT", [D, NB]),
        ("Wqs", [D, H * DK]),
        ("Wks", [D, H * DK]),
        ("Wvs", [D, H * DV]),
        ("Wqw", [D, H * DK]),
        ("Wfc", [D + H * DV, D]),
        ("Wfc1", [H * DV, D]),
    ]:
        dt_in[name] = nc.dram_tensor(name, shape, FR, kind="ExternalInput").ap()
    for name, shape in [
        ("bqsT", [128, 4]),
        ("bksT", [128, 4]),
        ("bqwT", [128, 4]),
        ("bkwT", [128, 4]),
        ("bvsT", [128, 4]),
        ("bfc1T", [128, 4]),
        ("bfcT", [128, 4]),
        ("bvw", [H * DV]),
    ]:
        dt_in[name] = nc.dram_tensor(name, shape, FP, kind="ExternalInput").ap()
    outT_d = nc.dram_tensor("outT", [D, LQ], FP, kind="ExternalOutput").ap()

    with tile.TileContext(nc) as tc:
        # ------------------------------------------------ persistent pools
        ppool_cm = tc.tile_pool(name="persist", bufs=1)
        ppool = ppool_cm.__enter__()
        scpool_cm = tc.tile_pool(name="scps", bufs=2, space="PSUM")
        scpool = scpool_cm.__enter__()
        vpool_cm = tc.tile_pool(name="valps", bufs=2, space="PSUM")
        vpool = vpool_cm.__enter__()
        ewpool_cm = tc.tile_pool(name="ewp", bufs=6)
        ewpool = ewpool_cm.__enter__()
        smpool_cm = tc.tile_pool(name="small", bufs=6)
        smpool = smpool_cm.__enter__()

        ident = ppool.tile([128, 128], FP, tag="ident")
        ident16 = ppool.tile([128, 128], FP16, tag="ident16")

        # persistent sbuf tensors
        qw_cmp = ppool.tile([128, 4, LQ], BF, tag="qw_cmp")
        ks_sb = ppool.tile([128, 4, NB], FR, tag="ks_sb")
        attn_sb = ppool.tile([128, 4, H, NB], FP, tag="attn_sb")
        fc1T_sb = ppool.tile([128, 4, LQ], FR, tag="fc1T")
        kw_sb = ppool.tile([128, 4, NTOK], BF, tag="kw_sb")
        vw_sb = ppool.tile([128, NTOK // 128, H, DV + 1], BF, tag="vw_sb")
        vs_sb = ppool.tile([NB, H * DV], FR, tag="vs_sb")
        Wkw_sb = ppool.tile([128, 4, H * DK], BF, tag="Wkw")
        Wvw_sb = ppool.tile([128, 4, H * DV], BF, tag="Wvw")
        ctx_sT = ppool.tile([128, 4, LQ], FR, tag="ctx_sT")

        stpool_cm = tc.tile_pool(name="stage", bufs=3)
        stpool = stpool_cm.__enter__()

        ctx_wT = ppool.tile([128, 4, LQ], FR, tag="ctx_wT")

        # ones column of vw_sb (denominator trick)
        nc.vector.memset(vw_sb[:, :, :, DV : DV + 1], 1.0)

        stg_cur = {}

        def kw_piece(sg, mo, j):
            """4 accumulating MMs -> [128,512] psum -> ACT copy to kw_sb."""
            if mo == 0 and j == 0:
                stg = stpool.tile([128, 4, 1024], BF, tag="stg", name="kwstg")
                nc.sync.dma_start(
                    out=stg[:], in_=_dram_chunks(dt_in["kwT"], 4, 1024, col_off=sg * 1024)
                )
                stg_cur["kw"] = stg
            stg = stg_cur["kw"]
            ps = scpool.tile([128, 2, 512], FP, tag="sc", name="kwps")
            for k in range(4):
                nc.tensor.matmul(
                    ps[:, 0, :],
                    Wkw_sb[:, k, mo * 128 : (mo + 1) * 128],
                    stg[:, k, j * 512 : (j + 1) * 512],
                    start=(k == 0),
                    stop=(k == 3),
                )
            # b_kw cancels in softmax/ratio; pure copy on ACT
            nc.scalar.activation(
                kw_sb[:, mo, sg * 1024 + j * 512 : sg * 1024 + (j + 1) * 512],
                ps[:, 0, :],
                ACTF.Copy,
            )

        def vw_piece(sg, tp, j):
            """One token-chunk (128 tok) of vw: 4 MMs + DVE bias-fold copy."""
            if tp == 0 and j == 0:
                stg = stpool.tile([128, 4, 1024], BF, tag="stg", name="vwstg")
                nc.sync.dma_start(
                    out=stg[:], in_=_dram_chunks(dt_in["vwT"], 4, 1024, col_off=sg * 1024)
                )
                stg_cur["vw"] = stg
            stg = stg_cur["vw"]
            tcl = tp * 2 + j
            ps = scpool.tile([128, 2, 512], FP, tag="sc", name="vwps")
            for k in range(4):
                nc.tensor.matmul(
                    ps[:, 0, :],
                    stg[:, k, tcl * 128 : (tcl + 1) * 128],
                    Wvw_sb[:, k, :],
                    start=(k == 0),
                    stop=(k == 3),
                )
            # vw copy with b_vw folded in: out[tok, (h,dv)] = ps + bvw
            tc0 = sg * 8 + tcl
            out_ap = _ax(vw_sb[:, tc0, 0, 0], [[DV + 1, H], [1, DV]])
            in0 = ps[:, 0, :].rearrange("p (h x) -> p h x", x=DV)
            in1 = _ax(bvw_bc, [[DV, H], [1, DV]])
            nc.vector.scalar_tensor_tensor(
                out_ap, in0, 1.0, in1, op0=ALU.mult, op1=ALU.add
            )

        PIECES = []
        for sg in range(4):
            for mo in range(4):
                for j in range(2):
                    PIECES.append((sg, "kw", mo, j))
            for tp in range(4):
                for j in range(2):
                    PIECES.append((sg, "vw", tp, j))

        def emit_piece(p):
            sg, kind, a, j = p
            if kind == "kw":
                kw_piece(sg, a, j)
            else:
                vw_piece(sg, a, j)

        # ------------------------------------------- phase 1+2: small branch
        with tc.tile_pool(name="phaseA", bufs=1) as apool, tc.tile_pool(
            name="wring", bufs=2
        ) as wpool:
            qT_sb = apool.tile([128, 4, LQ], FR, tag="qT_sb")
            ksT_sb = apool.tile([128, 4, NB], FR, tag="ksT_sb")
            vsT_sb = apool.tile([128, 4, NB], FR, tag="vsT_sb")
            qs_sb = apool.tile([128, 4, LQ], FR, tag="qs_sb")

            def load_W(wn, wdt=FR):
                t = wpool.tile([128, 4, D], wdt, tag="Wring", name=wn)
                nc.sync.dma_start(out=t[:], in_=_dram_chunks(dt_in[wn], 4, D))
                return t

            # critical-path DMAs first: qT + b_qw + Wqw feed the first matmuls
            nc.sync.dma_start(out=qT_sb[:], in_=_dram_chunks(dt_in["qT"], 4, LQ))
            btiles = {}
            for bn in ["bqwT"]:
                t = ppool.tile([128, 4], FP, tag=bn)
                nc.sync.dma_start(out=t[:], in_=dt_in[bn][:])
                btiles[bn] = t
            make_identity(nc, ident)
            make_identity(nc, ident16)

            # qw projection -> compact [128, hp, LQ] bf16 (head 2hp on 0:64,
            # head 2hp+1 on 64:128 partitions)
            Wqw_t = load_W("Wqw")
            for mo in range(4):
                ps = scpool.tile([128, 2, 512], FP, tag="sc", name="qwps")
                for k in range(4):
                    nc.tensor.matmul(
                        ps[:, 0, :],
                        Wqw_t[:, k, mo * 128 : (mo + 1) * 128],
                        qT_sb[:, k, :],
                        start=(k == 0),
                        stop=(k == 3),
                    )
                nc.vector.tensor_scalar_add(
                    qw_cmp[:, mo, :], ps[:, 0, :], btiles["bqwT"][:, mo : mo + 1]
                )
            nc.sync.dma_start(out=Wkw_sb[:], in_=_dram_chunks(dt_in["Wkw"], 4, H * DK))
            nc.sync.dma_start(out=Wvw_sb[:], in_=_dram_chunks(dt_in["Wvw"], 4, H * DV))
            # remaining phase-A DMAs
            nc.sync.dma_start(out=ksT_sb[:], in_=_dram_chunks(dt_in["ksT"], 4, NB))
            nc.sync.dma_start(out=vsT_sb[:], in_=_dram_chunks(dt_in["vsT"], 4, NB))
            for bn in ["bqsT", "bvsT", "bfc1T", "bfcT"]:
                t = ppool.tile([128, 4], FP, tag=bn)
                nc.sync.dma_start(out=t[:], in_=dt_in[bn][:])
                btiles[bn] = t
            bvw_bc = ppool.tile([128, H * DV], FP, tag="bvw_bc")
            src = dt_in["bvw"]
            bcast_ap = bass.AP(src.tensor, src.offset, [[0, 128]] + [list(x) for x in src.ap])
            nc.sync.dma_start(out=bvw_bc[:], in_=bcast_ap)
            for p in PIECES[0:16]:
                emit_piece(p)
            # qs projection
            Wqs_t = load_W("Wqs")
            for mo in range(4):
                ps = scpool.tile([128, 2, 512], FP, tag="sc", name="qsps")
                for k in range(4):
                    nc.tensor.matmul(
                        ps[:, 0, :],
                        Wqs_t[:, k, mo * 128 : (mo + 1) * 128],
                        qT_sb[:, k, :],
                        start=(k == 0),
                        stop=(k == 3),
                    )
                nc.vector.tensor_scalar_add(
                    qs_sb[:, mo, :], ps[:, 0, :], btiles["bqsT"][:, mo : mo + 1]
                )

            # ks projection (b_ks cancels in sentence softmax)
            Wks_t = load_W("Wks")
            for mo in range(4):
                ps = scpool.tile([128, 2, 512], FP, tag="sc", name="ksps")
                for k in range(4):
                    nc.tensor.matmul(
                        ps[:, 0, 0:NB],
                        Wks_t[:, k, mo * 128 : (mo + 1) * 128],
                        ksT_sb[:, k, :],
                        start=(k == 0),
                        stop=(k == 3),
                    )
                nc.scalar.activation(ks_sb[:, mo, :], ps[:, 0, 0:NB], ACTF.Copy)

            # vs projection: out [nb, hdv]
            Wvs_t = load_W("Wvs")
            ps = scpool.tile([128, 2, 512], FP, tag="sc", name="vsps")
            for k in range(4):
                nc.tensor.matmul(
                    ps[0:NB, 0, :],
                    vsT_sb[:, k, :],
                    Wvs_t[:, k, :],
                    start=(k == 0),
                    stop=(k == 3),
                )
            nc.scalar.activation(vs_sb[:, :], ps[0:NB, 0, :], ACTF.Copy)

            # sentence attention: scores + exp per head, then batched softmax
            ews_all = apool.tile([128, 4, H, NB], FP, tag="ews_all")
            for h in range(H):
                hp, po = h // 2, (h % 2) * 64
                ps = scpool.tile([128, 2, 512], FP, tag="sc", name="sattps")
                for qo in range(4):
                    nc.tensor.matmul(
                        ps[:, 0, qo * NB : (qo + 1) * NB],
                        qs_sb[po : po + 64, hp, qo * 128 : (qo + 1) * 128],
                        ks_sb[po : po + 64, hp, :],
                        start=True,
                        stop=True,
                    )
                nc.scalar.activation(
                    ews_all[:, :, h, :],
                    ps[:, 0, 0 : 4 * NB].rearrange("p (a x) -> p a x", x=NB),
                    ACTF.Exp,
                    scale=SCALE,
                )
            # batched softmax over nb: den[128, (qo,h)] -> recip -> mul
            den_t = apool.tile([128, 4, H], FP, tag="den_t")
            nc.vector.tensor_reduce(den_t[:], ews_all[:], AX, ALU.add)
            rec_t = apool.tile([128, 4, H], FP, tag="rec_t")
            nc.vector.reciprocal(rec_t[:], den_t[:])
            rec_bc = _ax(rec_t, [[H, 4], [1, H], [0, NB]])
            nc.vector.tensor_tensor(attn_sb[:], ews_all[:], rec_bc, op=ALU.mult)

        # ---------------- main loop: projections + token attention pipeline

        # per-iteration state for the 2-deep software pipeline
        ew_store = {}

        t1pool_cm = tc.tile_pool(name="tail1", bufs=1)
        t1pool = t1pool_cm.__enter__()
        aspool_cm = tc.tile_pool(name="asTring", bufs=2)
        aspool = aspool_cm.__enter__()
        accpool_cm = tc.tile_pool(name="accp", bufs=4)
        accpool = accpool_cm.__enter__()
        tmppool_cm = tc.tile_pool(name="tmpp", bufs=4)
        tmppool = tmppool_cm.__enter__()
        Wfc1_sb = t1pool.tile([128, 4, D], FR, tag="Wfc1l")
        nc.sync.dma_start(out=Wfc1_sb[:], in_=_dram_chunks(dt_in["Wfc1"], 4, D))

        def sentence_tail(h):
            hp, po = h // 2, (h % 2) * 64
            asT_h = aspool.tile([NB, 4, 128], FR, tag="asT", name="asT_h")
            psT = scpool.tile([128, 2, 512], FP, tag="sc", name="sattT")
            for qo in range(4):
                nc.tensor.transpose(
                    psT[0:NB, 0, qo * 128 : (qo + 1) * 128], attn_sb[:, qo, h, :], ident[:]
                )
            nc.scalar.activation(asT_h[:, :, :], psT[0:NB, 0, :], ACTF.Copy)
            ps = scpool.tile([128, 2, 512], FP, tag="sc", name="ctxsps")
            nc.tensor.matmul(
                ps[0:64, 0, :],
                vs_sb[:, h * 64 : (h + 1) * 64],
                asT_h[:, :, :],
                start=True,
                stop=True,
            )
            nc.vector.tensor_scalar_add(
                ctx_sT[po : po + 64, hp, :],
                ps[0:64, 0, :],
                btiles["bvsT"][po : po + 64, hp : hp + 1],
            )

        def emit_fc1():
            for mo in range(4):
                ps = scpool.tile([128, 2, 512], FP, tag="sc", name="fc1ps")
                for k in range(4):
                    nc.tensor.matmul(
                        ps[:, 0, :],
                        Wfc1_sb[:, k, mo * 128 : (mo + 1) * 128],
                        ctx_sT[:, k, :],
                        start=(k == 0),
                        stop=(k == 3),
                    )
                nc.vector.tensor_scalar(
                    fc1T_sb[:, mo, :],
                    ps[:, 0, :],
                    0.5,
                    btiles["bfc1T"][:, mo : mo + 1],
                    op0=ALU.mult,
                    op1=ALU.add,
                )

        acc_cur = [None] * 4

        def emit_scores(it):
            n, hp = divmod(it, 4)
            psA = scpool.tile([128, 2, 512], FP, tag="sc", name="psA")
            psB = scpool.tile([128, 2, 512], FP, tag="sc", name="psB")
            for tl in range(2):
                tcg = 2 * n + tl
                nc.tensor.matmul(
                    psA[:, tl, :],
                    kw_sb[0:64, hp, tcg * 128 : (tcg + 1) * 128],
                    qw_cmp[0:64, hp, :],
                    start=True,
                    stop=True,
                    tile_position=(0, 0),
                )
                nc.tensor.matmul(
                    psB[:, tl, :],
                    kw_sb[64:128, hp, tcg * 128 : (tcg + 1) * 128],
                    qw_cmp[64:128, hp, :],
                    start=True,
                    stop=True,
                    tile_position=(64, 0),
                )
            tiles = []
            for j, psx in enumerate((psA, psB)):
                ew_t = ewpool.tile([128, 2, 512], BF, tag="ew", name="ew_t")
                if (2 * it + j) in DVE_KS:
                    nc.vector.tensor_scalar(
                        ew_t.bitcast(I16)[:],
                        psx[:],
                        SCH_A * SCALE,
                        SCH_B,
                        op0=ALU.mult,
                        op1=ALU.add,
                    )
                else:
                    nc.scalar.activation(_flat2(ew_t), _flat2(psx), ACTF.Exp, scale=SCALE)
                tiles.append(ew_t)
            ew_store[it] = tiles
            if it in (5, 7, 9, 11, 13, 15, 17, 19):
                sentence_tail((it - 5) // 2)
            elif it == 21:
                emit_fc1()
            sg, k = it // 16 + 1, it % 16
            if sg <= 3:
                emit_piece(PIECES[sg * 16 + k])

        def emit_s3(it):
            n, hp = divmod(it, 4)
            tiles = ew_store.pop(it)
            # per-iteration vals tile: 8 runs of 65 at stride 128 (2 banks);
            # col 64 = denominator (ones column of vw)
            vals = vpool.tile([128, 8, 128], FP, tag="vals", name="vals")
            for hl in range(2):
                h = 2 * hp + hl
                ew_t = tiles[hl]
                for qo in range(4):
                    hq = hl * 4 + qo
                    for tl in range(2):
                        tcg = 2 * n + tl
                        nc.tensor.matmul(
                            vals[:, hq, 0 : DV + 1],
                            ew_t[:, tl, qo * 128 : (qo + 1) * 128],
                            vw_sb[:, tcg, h, :],
                            start=(tl == 0),
                            stop=(tl == 1),
                        )
            if n == 0:
                acc_cur[hp] = accpool.tile([128, 4, 2, DV], FP16, tag="acc", name="acc")
            acc = acc_cur[hp]
            # fac[hl,qo] = attn_s * (1/den)
            rec = smpool.tile([128, 2, 4], FP, tag="rec")
            den_ap = _ax(vals[:, 0, DV], [[512, 2], [128, 4]])
            nc.vector.reciprocal(rec[:], den_ap)
            fac = smpool.tile([128, 2, 4], FP, tag="fac")
            attn_ap = _ax(attn_sb[:, 0, 2 * hp, n], [[NB, 2], [H * NB, 4]])
            nc.vector.tensor_tensor(fac[:], rec[:], attn_ap, op=ALU.mult)
            # acc (+)= vals * fac ; iterate (hl, qo, dv), acc laid out (qo, hl, dv)
            in0 = _ax(vals[:, 0, 0], [[512, 2], [128, 4], [1, DV]])
            fac_bc = _ax(fac[:, 0, 0], [[4, 2], [1, 4], [0, DV]])
            acc_w = _ax(acc[:, 0, 0, 0], [[DV, 2], [2 * DV, 4], [1, DV]])
            if n == 0:
                nc.vector.tensor_tensor(acc_w, in0, fac_bc, op=ALU.mult)
            else:
                tmp = tmppool.tile([128, 4, 2, DV], FP16, tag="tmp", name="tmp")
                tmp_w = _ax(tmp[:, 0, 0, 0], [[DV, 2], [2 * DV, 4], [1, DV]])
                nc.vector.tensor_tensor(tmp_w, in0, fac_bc, op=ALU.mult)
                nc.vector.tensor_tensor(acc[:], tmp[:], acc[:], op=ALU.add)
            if n == 15:
                # transpose ctx_w pair -> ctx_wT[:, hp, :]
                ps = scpool.tile([128, 2, 512], FP, tag="sc", name="ctps")
                psh = ps.bitcast(FP16)  # [128, 2, 1024] fp16 view
                for qo in range(4):
                    nc.tensor.transpose(
                        psh[:, 0, qo * 128 : (qo + 1) * 128],
                        acc[:, qo, :, :].rearrange("p a b -> p (a b)"),
                        ident16[:],
                    )
                nc.scalar.activation(ctx_wT[:, hp, :], psh[:, 0, 0:512], ACTF.Copy)

        for it in range(64 + 3):
            if it < 64:
                emit_scores(it)
            if it >= 3:
                emit_s3(it - 3)

        tmppool_cm.__exit__(None, None, None)
        accpool_cm.__exit__(None, None, None)
        aspool_cm.__exit__(None, None, None)
        t1pool_cm.__exit__(None, None, None)
        stpool_cm.__exit__(None, None, None)

        # ---------------- final fc
        lpool_cm = tc.tile_pool(name="late", bufs=1)
        lpool = lpool_cm.__enter__()
        outT_sb = lpool.tile([128, 4, LQ], FP, tag="outT_sb")
        Wfc_sb = lpool.tile([128, 8, D], FR, tag="Wfc")
        nc.sync.dma_start(out=Wfc_sb[:], in_=_dram_chunks(dt_in["Wfc"], 8, D))
        for mo in range(4):
            ps = scpool.tile([128, 2, 512], FP, tag="sc", name="fcps")
            for cc in range(4):
                nc.tensor.matmul(
                    ps[:, 0, :],
                    Wfc_sb[:, cc, mo * 128 : (mo + 1) * 128],
                    fc1T_sb[:, cc, :],
                    start=(cc == 0),
                    stop=False,
                )
            for cc in range(4):
                nc.tensor.matmul(
                    ps[:, 0, :],
                    Wfc_sb[:, 4 + cc, mo * 128 : (mo + 1) * 128],
                    ctx_wT[:, cc, :],
                    start=False,
                    stop=(cc == 3),
                )
            nc.vector.tensor_scalar_add(
                outT_sb[:, mo, :], ps[:, 0, :], btiles["bfcT"][:, mo : mo + 1]
            )
            nc.sync.dma_start(out=outT_d[mo * 128 : (mo + 1) * 128, :], in_=outT_sb[:, mo, :])

        lpool_cm.__exit__(None, None, None)
        smpool_cm.__exit__(None, None, None)
        ewpool_cm.__exit__(None, None, None)
        vpool_cm.__exit__(None, None, None)
        scpool_cm.__exit__(None, None, None)
        ppool_cm.__exit__(None, None, None)

    ns = _split_multi_waits(nc)
    print(f"[kernel] split {ns} extra sem waits onto NOPs", file=sys.stderr)
    return nc


_NC_CACHE = None


def _get_nc():
    global _NC_CACHE
    if _NC_CACHE is None:
        _NC_CACHE = build_program()
    return _NC_CACHE


def make_in_maps(inputs):
    f = lambda x: np.ascontiguousarray(np.asarray(x, dtype=np.float32))
    q, k_w, v_w, k_s, v_s = (f(inputs[n]) for n in ["q", "k_w", "v_w", "k_s", "v_s"])
    W = {n: f(inputs[n]) for n in inputs if n.startswith(("W_", "b_"))}

    def bT(v, scale=1.0):
        return np.ascontiguousarray((v * scale).reshape(4, 128).T)

    bf = ml_dtypes.bfloat16
    shared = {
        "Wqs": W["W_qs"], "Wks": W["W_ks"], "Wvs": W["W_vs"],
        "Wqw": W["W_qw"], "Wkw": W["W_kw"].astype(bf), "Wvw": W["W_vw"].astype(bf),
        "Wfc1": W["W_fc1"], "Wfc": W["W_fc"],
        "bqsT": bT(W["b_qs"]), "bksT": bT(W["b_ks"]), "bqwT": bT(W["b_qw"]),
        "bkwT": bT(W["b_kw"]), "bvsT": bT(W["b_vs"]),
        "bfc1T": bT(W["b_fc1"], 0.5), "bfcT": bT(W["b_fc"], 0.5),
        "bvw": W["b_vw"],
    }
    in_maps = []
    for c in range(N_CORES):
        b, half = divmod(c, 2)
        blk = slice(half * NBH, half * NBH + NBH)
        ks_r = np.roll(k_s[b], -half * NBH, axis=0)
        vs_r = np.roll(v_s[b], -half * NBH, axis=0)
        m = dict(shared)
        m["qT"] = np.ascontiguousarray(q[b].T)
        m["kwT"] = np.ascontiguousarray(k_w[b, blk].reshape(NTOK, D).T.astype(bf))
        m["vwT"] = np.ascontiguousarray(v_w[b, blk].reshape(NTOK, D).T.astype(bf))
        m["ksT"] = np.ascontiguousarray(ks_r.T)
        m["vsT"] = np.ascontiguousarray(vs_r.T)
        in_maps.append(m)
    return in_maps


def run_cores(inputs, trace=False):
    nc = _get_nc()
    in_maps = make_in_maps(inputs)
    res = run_bass_kernel_spmd(nc, in_maps, list(range(N_CORES)), trace=trace)
    return res


def assemble(res):
    out = np.empty((B, LQ, D), dtype=np.float32)
    for b in range(B):
        out[b] = (res.results[2 * b]["outT"] + res.results[2 * b + 1]["outT"]).T
    return out


def kernel(**inputs) -> np.ndarray:
    res = run_cores(inputs, trace=False)
    return assemble(res)


if __name__ == "__main__":
    import reference

    inp = {k: np.asarray(v) for k, v in reference.setup_inputs().items()}
    out = kernel(**inp)
    exp = np.asarray(reference.reference(**inp))
    err = np.abs(out - exp).max() / np.abs(exp).max()
    print("max rel err:", err)


# revision 31
# speedup vs baseline: 1.0113x; 1.0012x over previous
"""
Trainium2 Bass kernel for nn_MultiHeadHierarchicalAttention (v2).

Sharding: 8 cores = (batch b in 0..3) x (block-half in 0..1), 16 blocks/core.
The sentence branch is computed redundantly on both cores of a batch (scaled
0.5); the host sums the two per-batch partial outputs.

v2 restructure vs baseline:
  * token scores run as row-tiled CONCURRENT K=64 matmul pairs (head 2hp on
    PE rows 0:63, head 2hp+1 on 64:127) -> half the PE score cycles.
  * exp is split between ACT (table exp) and DVE (Schraudolph int16->bf16
    bitcast exp, max rel err ~3%) with a tunable tile quota.
  * the per-block sentence/denominator factor is applied batched: S3 writes
    65-wide runs (64 ctx cols + ones-column denom) at stride 128 into a
    4-bank PSUM tile; DVE does recip + factor + one broadcast TT-mul per
    2-block group into a bf16 stash [hq, dv, gn], then ONE reduce per
    head-pair produces ctx directly. Replaces 512 tiny STT ops.
  * b_vw is folded into vw_sb during the projection copy (TT-add with a
    broadcast bias tile), removing the per-head bias pass.
"""

import sys

sys.path.insert(0, "/opt/trn_rl_repo")

import numpy as np
import ml_dtypes
import concourse.bass as bass
import concourse.tile as tile
from concourse import mybir
from concourse.bass_utils import run_bass_kernel_spmd
from concourse.vector_clock import ScopedClock
from concourse.masks import make_identity

# ---------------------------------------------------------------- constants
B, LQ, NB, NT = 4, 512, 32, 256
D, H, DK, DV = 512, 8, 64, 64
NBH = NB // 2  # blocks per core
NTOK = NBH * NT  # tokens per core = 4096
SCALE = 0.125
FP = mybir.dt.float32
FR = mybir.dt.float32r
BF = mybir.dt.bfloat16
I16 = mybir.dt.int16
N_CORES = 8

AX = mybir.AxisListType.X
ALU = mybir.AluOpType
ACTF = mybir.ActivationFunctionType

# Schraudolph bf16 exp constants: int16(x*A + B) bitcast to bf16 ~ exp(x)
SCH_A = float((2.0**7) / np.log(2.0))
SCH_B = float(16256 - 5.5)
# how many of the 128 exp tiles run on DVE instead of ACT (load balance)
N_DVE_EXP = 24
DVE_KS = {8 + int(round(i * 119.0 / (N_DVE_EXP - 1))) for i in range(N_DVE_EXP)}
FP16 = mybir.dt.float16
F8 = mybir.dt.float8e4
KW_UP = 64.0  # host scales W_kw/W_vw by 64 so fp8 weights aren't subnormal


# --------------------------------------------------------- drain workaround
def _patched_drain_and_barrier(self, tick_clock, wait_clock):
    nop_inst = self.nc.sync.nop(nofuse=True)
    wait_clock.add_sem_waits(nop_inst.ins, ScopedClock({None: tick_clock.global_clock}))
    waits = list(nop_inst.ins.sync_info.on_wait or [])
    if len(waits) > 1:
        nop_inst.ins.sync_info.on_wait = waits[:1]
        rest = waits[1:]
        while rest:
            extra = self.nc.sync.nop(nofuse=True)
            if extra.ins.sync_info is None:
                extra.ins.sync_info = mybir.SyncInfo(on_wait=[], on_update=[])
            extra.ins.sync_info.on_wait = rest[:1]
            rest = rest[1:]
    self.nc.sync.drain()
    self.nc.all_engine_barrier()
    assert self.sems is not None
    popped = self.nc._tile_sem_poison_stack.pop()
    assert popped is self._sem_poison
    self.nc.clear_and_free_semaphores(list(self.sems.allocated().values()))
    self.nc.all_engine_barrier()


tile.TileContext._drain_and_barrier = _patched_drain_and_barrier


def _r(ap):
    return ap.bitcast(mybir.dt.float32r)


_NO_SPLIT_OPCODES = {
    "CollectiveCompute",
    "EventSemaphore",
}
_split_counter = [0]


def _split_multi_waits(nc):
    n_split = 0
    for fn in nc.m.functions:
        for bb in fn.blocks:
            changed = False
            out = []
            for inst in bb.instructions:
                si = inst.sync_info
                if (
                    si is not None
                    and si.on_wait
                    and len(list(si.on_wait)) > 1
                    and inst.opcode not in _NO_SPLIT_OPCODES
                ):
                    waits = list(si.on_wait)
                    for w in waits[:-1]:
                        _split_counter[0] += 1
                        nop = mybir.InstNoOp(name=f"I-wsplit-{_split_counter[0]}")
                        nop.engine = inst.engine
                        nop.sync_info = mybir.SyncInfo(on_wait=[w], on_update=[])
                        out.append(nop)
                        n_split += 1
                    si.on_wait = waits[-1:]
                    changed = True
                out.append(inst)
            if changed:
                bb.instructions = out
    return n_split


def _flat2(ap):
    return ap.rearrange("p a b -> p (a b)")


def _ax(ap, axes):
    """Build an AP on the same tensor with explicit [stride, num] free axes."""
    return bass.AP(ap.tensor, ap.offset, [list(ap.ap[0])] + [list(a) for a in axes])



def _dram_chunks(dram_ap, nk, ncols, col_off=0, row_w=None):
    """AP reading nk row-chunks of 128 from a [nk*128, W] dram tensor as
    [128 part, nk, ncols]."""
    W = row_w if row_w is not None else dram_ap.ap[-1][1]
    return bass.AP(
        dram_ap.tensor,
        dram_ap.offset + col_off,
        [[W, 128], [128 * W, nk], [1, ncols]],
    )


# ------------------------------------------------------------ program build
def build_program():
    nc = bass.Bass("TRN2", target_bir_lowering=False, debug=False, num_devices=N_CORES)

    dt_in = {}
    for name, shape in [
        ("kwT", [D, NTOK]),
        ("vwT", [D, NTOK]),
        ("Wkw", [D, H * DK]),
        ("Wvw", [D, H * DV]),
    ]:
        dt_in[name] = nc.dram_tensor(name, shape, BF, kind="ExternalInput").ap()
    for name, shape in [
        ("qT", [D, LQ]),
        ("ksT", [D, NB]),
        ("vsT", [D, NB]),
        ("Wqs", [D, H * DK]),
        ("Wks", [D, H * DK]),
        ("Wvs", [D, H * DV]),
        ("Wqw", [D, H * DK]),
        ("Wfc", [D + H * DV, D]),
        ("Wfc1", [H * DV, D]),
    ]:
        dt_in[name] = nc.dram_tensor(name, shape, FR, kind="ExternalInput").ap()
    for name, shape in [
        ("bqsT", [128, 4]),
        ("bksT", [128, 4]),
        ("bqwT", [128, 4]),
        ("bkwT", [128, 4]),
        ("bvsT", [128, 4]),
        ("bfc1T", [128, 4]),
        ("bfcT", [128, 4]),
        ("bvw", [H * DV]),
    ]:
        dt_in[name] = nc.dram_tensor(name, shape, FP, kind="ExternalInput").ap()
    outT_d = nc.dram_tensor("outT", [D, LQ], FP, kind="ExternalOutput").ap()

    with tile.TileContext(nc) as tc:
        # ------------------------------------------------ persistent pools
        ppool_cm = tc.tile_pool(name="persist", bufs=1)
        ppool = ppool_cm.__enter__()
        scpool_cm = tc.tile_pool(name="scps", bufs=2, space="PSUM")
        scpool = scpool_cm.__enter__()
        vpool_cm = tc.tile_pool(name="valps", bufs=2, space="PSUM")
        vpool = vpool_cm.__enter__()
        ewpool_cm = tc.tile_pool(name="ewp", bufs=6)
        ewpool = ewpool_cm.__enter__()
        smpool_cm = tc.tile_pool(name="small", bufs=6)
        smpool = smpool_cm.__enter__()

        ident = ppool.tile([128, 128], FP, tag="ident")
        ident16 = ppool.tile([128, 128], FP16, tag="ident16")

        # persistent sbuf tensors
        qw_cmp = ppool.tile([128, 4, LQ], BF, tag="qw_cmp")
        ks_sb = ppool.tile([128, 4, NB], FR, tag="ks_sb")
        attn_sb = ppool.tile([128, 4, H, NB], FP, tag="attn_sb")
        fc1T_sb = ppool.tile([128, 4, LQ], FR, tag="fc1T")
        kw_sb = ppool.tile([128, 4, NTOK], BF, tag="kw_sb")
        vw_sb = ppool.tile([128, NTOK // 128, H, DV + 1], BF, tag="vw_sb")
        vs_sb = ppool.tile([NB, H * DV], FR, tag="vs_sb")
        Wkw_sb = ppool.tile([128, 4, H * DK], BF, tag="Wkw")
        Wvw_sb = ppool.tile([128, 4, H * DV], BF, tag="Wvw")
        ctx_sT = ppool.tile([128, 4, LQ], FR, tag="ctx_sT")

        stpool_cm = tc.tile_pool(name="stage", bufs=3)
        stpool = stpool_cm.__enter__()

        ctx_wT = ppool.tile([128, 4, LQ], FR, tag="ctx_wT")

        # ones column of vw_sb (denominator trick)
        nc.vector.memset(vw_sb[:, :, :, DV : DV + 1], 1.0)

        stg_cur = {}

        def kw_piece(sg, mo, j):
            """4 accumulating MMs -> [128,512] psum -> ACT copy to kw_sb."""
            if mo == 0 and j == 0:
                stg = stpool.tile([128, 4, 1024], BF, tag="stg", name="kwstg")
                nc.sync.dma_start(
                    out=stg[:], in_=_dram_chunks(dt_in["kwT"], 4, 1024, col_off=sg * 1024)
                )
                stg_cur["kw"] = stg
            stg = stg_cur["kw"]
            ps = scpool.tile([128, 2, 512], FP, tag="sc", name="kwps")
            for k in range(4):
                nc.tensor.matmul(
                    ps[:, 0, :],
                    Wkw_sb[:, k, mo * 128 : (mo + 1) * 128],
                    stg[:, k, j * 512 : (j + 1) * 512],
                    start=(k == 0),
                    stop=(k == 3),
                )
            # b_kw cancels in softmax/ratio; pure copy on ACT
            nc.scalar.activation(
                kw_sb[:, mo, sg * 1024 + j * 512 : sg * 1024 + (j + 1) * 512],
                ps[:, 0, :],
                ACTF.Copy,
            )

        def vw_piece(sg, tp, j):
            """One token-chunk (128 tok) of vw: 4 MMs + DVE bias-fold copy."""
            if tp == 0 and j == 0:
                stg = stpool.tile([128, 4, 1024], BF, tag="stg", name="vwstg")
                nc.sync.dma_start(
                    out=stg[:], in_=_dram_chunks(dt_in["vwT"], 4, 1024, col_off=sg * 1024)
                )
                stg_cur["vw"] = stg
            stg = stg_cur["vw"]
            tcl = tp * 2 + j
            ps = scpool.tile([128, 2, 512], FP, tag="sc", name="vwps")
            for k in range(4):
                nc.tensor.matmul(
                    ps[:, 0, :],
                    stg[:, k, tcl * 128 : (tcl + 1) * 128],
                    Wvw_sb[:, k, :],
                    start=(k == 0),
                    stop=(k == 3),
                )
            # vw copy with b_vw folded in: out[tok, (h,dv)] = ps + bvw
            tc0 = sg * 8 + tcl
            out_ap = _ax(vw_sb[:, tc0, 0, 0], [[DV + 1, H], [1, DV]])
            in0 = ps[:, 0, :].rearrange("p (h x) -> p h x", x=DV)
            in1 = _ax(bvw_bc, [[DV, H], [1, DV]])
            nc.vector.scalar_tensor_tensor(
                out_ap, in0, 1.0, in1, op0=ALU.mult, op1=ALU.add
            )

        PIECES = []
        for sg in range(4):
            for mo in range(4):
                for j in range(2):
                    PIECES.append((sg, "kw", mo, j))
            for tp in range(4):
                for j in range(2):
                    PIECES.append((sg, "vw", tp, j))

        def emit_piece(p):
            sg, kind, a, j = p
            if kind == "kw":
                kw_piece(sg, a, j)
            else:
                vw_piece(sg, a, j)

        # ------------------------------------------- phase 1+2: small branch
        with tc.tile_pool(name="phaseA", bufs=1) as apool, tc.tile_pool(
            name="wring", bufs=2
        ) as wpool:
            qT_sb = apool.tile([128, 4, LQ], FR, tag="qT_sb")
            ksT_sb = apool.tile([128, 4, NB], FR, tag="ksT_sb")
            vsT_sb = apool.tile([128, 4, NB], FR, tag="vsT_sb")
            qs_sb = apool.tile([128, 4, LQ], FR, tag="qs_sb")

            def load_W(wn, wdt=FR):
                t = wpool.tile([128, 4, D], wdt, tag="Wring", name=wn)
                nc.sync.dma_start(out=t[:], in_=_dram_chunks(dt_in[wn], 4, D))
                return t

            # critical-path DMAs first: qT + b_qw + Wqw feed the first matmuls
            nc.sync.dma_start(out=qT_sb[:], in_=_dram_chunks(dt_in["qT"], 4, LQ))
            btiles = {}
            for bn in ["bqwT"]:
                t = ppool.tile([128, 4], FP, tag=bn)
                nc.sync.dma_start(out=t[:], in_=dt_in[bn][:])
                btiles[bn] = t
            make_identity(nc, ident)
            make_identity(nc, ident16)

            # qw projection -> compact [128, hp, LQ] bf16 (head 2hp on 0:64,
            # head 2hp+1 on 64:128 partitions)
            Wqw_t = load_W("Wqw")
            for mo in range(4):
                ps = scpool.tile([128, 2, 512], FP, tag="sc", name="qwps")
                for k in range(4):
                    nc.tensor.matmul(
                        ps[:, 0, :],
                        Wqw_t[:, k, mo * 128 : (mo + 1) * 128],
                        qT_sb[:, k, :],
                        start=(k == 0),
                        stop=(k == 3),
                    )
                nc.vector.tensor_scalar_add(
                    qw_cmp[:, mo, :], ps[:, 0, :], btiles["bqwT"][:, mo : mo + 1]
                )
            nc.sync.dma_start(out=Wkw_sb[:], in_=_dram_chunks(dt_in["Wkw"], 4, H * DK))
            nc.sync.dma_start(out=Wvw_sb[:], in_=_dram_chunks(dt_in["Wvw"], 4, H * DV))
            # remaining phase-A DMAs
            nc.sync.dma_start(out=ksT_sb[:], in_=_dram_chunks(dt_in["ksT"], 4, NB))
            nc.sync.dma_start(out=vsT_sb[:], in_=_dram_chunks(dt_in["vsT"], 4, NB))
            for bn in ["bqsT", "bvsT", "bfc1T", "bfcT"]:
                t = ppool.tile([128, 4], FP, tag=bn)
                nc.sync.dma_start(out=t[:], in_=dt_in[bn][:])
                btiles[bn] = t
            bvw_bc = ppool.tile([128, H * DV], FP, tag="bvw_bc")
            src = dt_in["bvw"]
            bcast_ap = bass.AP(src.tensor, src.offset, [[0, 128]] + [list(x) for x in src.ap])
            nc.sync.dma_start(out=bvw_bc[:], in_=bcast_ap)
            for p in PIECES[0:16]:
                emit_piece(p)
            # qs projection
            Wqs_t = load_W("Wqs")
            for mo in range(4):
                ps = scpool.tile([128, 2, 512], FP, tag="sc", name="qsps")
                for k in range(4):
                    nc.tensor.matmul(
                        ps[:, 0, :],
                        Wqs_t[:, k, mo * 128 : (mo + 1) * 128],
                        qT_sb[:, k, :],
                        start=(k == 0),
                        stop=(k == 3),
                    )
                nc.vector.tensor_scalar_add(
                    qs_sb[:, mo, :], ps[:, 0, :], btiles["bqsT"][:, mo : mo + 1]
                )

            # ks projection (b_ks cancels in sentence softmax)
            Wks_t = load_W("Wks")
            for mo in range(4):
                ps = scpool.tile([128, 2, 512], FP, tag="sc", name="ksps")
                for k in range(4):
                    nc.tensor.matmul(
                        ps[:, 0, 0:NB],
                        Wks_t[:, k, mo * 128 : (mo + 1) * 128],
                        ksT_sb[:, k, :],
                        start=(k == 0),
                        stop=(k == 3),
                    )
                nc.scalar.activation(ks_sb[:, mo, :], ps[:, 0, 0:NB], ACTF.Copy)

            # vs projection: out [nb, hdv]
            Wvs_t = load_W("Wvs")
            ps = scpool.tile([128, 2, 512], FP, tag="sc", name="vsps")
            for k in range(4):
                nc.tensor.matmul(
                    ps[0:NB, 0, :],
                    vsT_sb[:, k, :],
                    Wvs_t[:, k, :],
                    start=(k == 0),
                    stop=(k == 3),
                )
            nc.scalar.activation(vs_sb[:, :], ps[0:NB, 0, :], ACTF.Copy)

            # sentence attention: scores + exp per head, then batched softmax
            ews_all = apool.tile([128, 4, H, NB], FP, tag="ews_all")
            for h in range(H):
                hp, po = h // 2, (h % 2) * 64
                ps = scpool.tile([128, 2, 512], FP, tag="sc", name="sattps")
                for qo in range(4):
                    nc.tensor.matmul(
                        ps[:, 0, qo * NB : (qo + 1) * NB],
                        qs_sb[po : po + 64, hp, qo * 128 : (qo + 1) * 128],
                        ks_sb[po : po + 64, hp, :],
                        start=True,
                        stop=True,
                    )
                nc.scalar.activation(
                    ews_all[:, :, h, :],
                    ps[:, 0, 0 : 4 * NB].rearrange("p (a x) -> p a x", x=NB),
                    ACTF.Exp,
                    scale=SCALE,
                )
            # batched softmax over nb: den[128, (qo,h)] -> recip -> mul
            den_t = apool.tile([128, 4, H], FP, tag="den_t")
            nc.vector.tensor_reduce(den_t[:], ews_all[:], AX, ALU.add)
            rec_t = apool.tile([128, 4, H], FP, tag="rec_t")
            nc.vector.reciprocal(rec_t[:], den_t[:])
            rec_bc = _ax(rec_t, [[H, 4], [1, H], [0, NB]])
            nc.vector.tensor_tensor(attn_sb[:], ews_all[:], rec_bc, op=ALU.mult)

        # ---------------- main loop: projections + token attention pipeline

        # per-iteration state for the 2-deep software pipeline
        ew_store = {}

        t1pool_cm = tc.tile_pool(name="tail1", bufs=1)
        t1pool = t1pool_cm.__enter__()
        aspool_cm = tc.tile_pool(name="asTring", bufs=2)
        aspool = aspool_cm.__enter__()
        accpool_cm = tc.tile_pool(name="accp", bufs=4)
        accpool = accpool_cm.__enter__()
        tmppool_cm = tc.tile_pool(name="tmpp", bufs=4)
        tmppool = tmppool_cm.__enter__()
        Wfc1_sb = t1pool.tile([128, 4, D], FR, tag="Wfc1l")
        nc.sync.dma_start(out=Wfc1_sb[:], in_=_dram_chunks(dt_in["Wfc1"], 4, D))

        def sentence_tail(h):
            hp, po = h // 2, (h % 2) * 64
            asT_h = aspool.tile([NB, 4, 128], FR, tag="asT", name="asT_h")
            psT = scpool.tile([128, 2, 512], FP, tag="sc", name="sattT")
            for qo in range(4):
                nc.tensor.transpose(
                    psT[0:NB, 0, qo * 128 : (qo + 1) * 128], attn_sb[:, qo, h, :], ident[:]
                )
            nc.scalar.activation(asT_h[:, :, :], psT[0:NB, 0, :], ACTF.Copy)
            ps = scpool.tile([128, 2, 512], FP, tag="sc", name="ctxsps")
            nc.tensor.matmul(
                ps[0:64, 0, :],
                vs_sb[:, h * 64 : (h + 1) * 64],
                asT_h[:, :, :],
                start=True,
                stop=True,
            )
            nc.vector.tensor_scalar_add(
                ctx_sT[po : po + 64, hp, :],
                ps[0:64, 0, :],
                btiles["bvsT"][po : po + 64, hp : hp + 1],
            )

        def emit_fc1():
            for mo in range(4):
                ps = scpool.tile([128, 2, 512], FP, tag="sc", name="fc1ps")
                for k in range(4):
                    nc.tensor.matmul(
                        ps[:, 0, :],
                        Wfc1_sb[:, k, mo * 128 : (mo + 1) * 128],
                        ctx_sT[:, k, :],
                        start=(k == 0),
                        stop=(k == 3),
                    )
                nc.vector.tensor_scalar(
                    fc1T_sb[:, mo, :],
                    ps[:, 0, :],
                    0.5,
                    btiles["bfc1T"][:, mo : mo + 1],
                    op0=ALU.mult,
                    op1=ALU.add,
                )

        acc_cur = [None] * 4

        def emit_scores(it):
            n, hp = divmod(it, 4)
            psA = scpool.tile([128, 2, 512], FP, tag="sc", name="psA")
            psB = scpool.tile([128, 2, 512], FP, tag="sc", name="psB")
            for tl in range(2):
                tcg = 2 * n + tl
                nc.tensor.matmul(
                    psA[:, tl, :],
                    kw_sb[0:64, hp, tcg * 128 : (tcg + 1) * 128],
                    qw_cmp[0:64, hp, :],
                    start=True,
                    stop=True,
                    tile_position=(0, 0),
                )
                nc.tensor.matmul(
                    psB[:, tl, :],
                    kw_sb[64:128, hp, tcg * 128 : (tcg + 1) * 128],
                    qw_cmp[64:128, hp, :],
                    start=True,
                    stop=True,
                    tile_position=(64, 0),
                )
            tiles = []
            for j, psx in enumerate((psA, psB)):
                ew_t = ewpool.tile([128, 2, 512], BF, tag="ew", name="ew_t")
                if (2 * it + j) in DVE_KS:
                    nc.vector.tensor_scalar(
                        ew_t.bitcast(I16)[:],
                        psx[:],
                        SCH_A * SCALE,
                        SCH_B,
                        op0=ALU.mult,
                        op1=ALU.add,
                    )
                else:
                    nc.scalar.activation(_flat2(ew_t), _flat2(psx), ACTF.Exp, scale=SCALE)
                tiles.append(ew_t)
            ew_store[it] = tiles
            if it in (5, 7, 9, 11, 13, 15, 17, 19):
                sentence_tail((it - 5) // 2)
            elif it == 21:
                emit_fc1()
            sg, k = it // 16 + 1, it % 16
            if sg <= 3:
                emit_piece(PIECES[sg * 16 + k])

        def emit_s3(it):
            n, hp = divmod(it, 4)
            tiles = ew_store.pop(it)
            # per-iteration vals tile: 8 runs of 65 at stride 128 (2 banks);
            # col 64 = denominator (ones column of vw)
            vals = vpool.tile([128, 8, 128], FP, tag="vals", name="vals")
            for hl in range(2):
                h = 2 * hp + hl
                ew_t = tiles[hl]
                for qo in range(4):
                    hq = hl * 4 + qo
                    for tl in range(2):
                        tcg = 2 * n + tl
                        nc.tensor.matmul(
                            vals[:, hq, 0 : DV + 1],
                            ew_t[:, tl, qo * 128 : (qo + 1) * 128],
                            vw_sb[:, tcg, h, :],
                            start=(tl == 0),
                            stop=(tl == 1),
                        )
            if n == 0:
                acc_cur[hp] = accpool.tile([128, 4, 2, DV], FP16, tag="acc", name="acc")
            acc = acc_cur[hp]
            # fac[hl,qo] = attn_s * (1/den)
            rec = smpool.tile([128, 2, 4], FP, tag="rec")
            den_ap = _ax(vals[:, 0, DV], [[512, 2], [128, 4]])
            nc.vector.reciprocal(rec[:], den_ap)
            fac = smpool.tile([128, 2, 4], FP, tag="fac")
            attn_ap = _ax(attn_sb[:, 0, 2 * hp, n], [[NB, 2], [H * NB, 4]])
            nc.vector.tensor_tensor(fac[:], rec[:], attn_ap, op=ALU.mult)
            # acc (+)= vals * fac ; iterate (hl, qo, dv), acc laid out (qo, hl, dv)
            in0 = _ax(vals[:, 0, 0], [[512, 2], [128, 4], [1, DV]])
            fac_bc = _ax(fac[:, 0, 0], [[4, 2], [1, 4], [0, DV]])
            acc_w = _ax(acc[:, 0, 0, 0], [[DV, 2], [2 * DV, 4], [1, DV]])
            if n == 0:
                nc.vector.tensor_tensor(acc_w, in0, fac_bc, op=ALU.mult)
            else:
                tmp = tmppool.tile([128, 4, 2, DV], FP16, tag="tmp", name="tmp")
                tmp_w = _ax(tmp[:, 0, 0, 0], [[DV, 2], [2 * DV, 4], [1, DV]])
                nc.vector.tensor_tensor(tmp_w, in0, fac_bc, op=ALU.mult)
                nc.vector.tensor_tensor(acc[:], tmp[:], acc[:], op=ALU.add)
            if n == 15:
                # transpose ctx_w pair -> ctx_wT[:, hp, :]
                ps = scpool.tile([128, 2, 512], FP, tag="sc", name="ctps")
                psh = ps.bitcast(FP16)  # [128, 2, 1024] fp16 view
                for qo in range(4):
                    nc.tensor.transpose(
                        psh[:, 0, qo * 128 : (qo + 1) * 128],
                        acc[:, qo, :, :].rearrange("p a b -> p (a b)"),
                        ident16[:],
                    )
                nc.scalar.activation(ctx_wT[:, hp, :], psh[:, 0, 0:512], ACTF.Copy)

        for it in range(64 + 2):
            if it < 64:
                emit_scores(it)
            if it >= 2:
                emit_s3(it - 2)

        tmppool_cm.__exit__(None, None, None)
        accpool_cm.__exit__(None, None, None)
        aspool_cm.__exit__(None, None, None)
        t1pool_cm.__exit__(None, None, None)
        stpool_cm.__exit__(None, None, None)

        # ---------------- final fc
        lpool_cm = tc.tile_pool(name="late", bufs=1)
        lpool = lpool_cm.__enter__()
        outT_sb = lpool.tile([128, 4, LQ], FP, tag="outT_sb")
        Wfc_sb = lpool.tile([128, 8, D], FR, tag="Wfc")
        nc.sync.dma_start(out=Wfc_sb[:], in_=_dram_chunks(dt_in["Wfc"], 8, D))
        for mo in range(4):
            ps = scpool.tile([128, 2, 512], FP, tag="sc", name="fcps")
            for cc in range(4):
                nc.tensor.matmul(
                    ps[:, 0, :],
                    Wfc_sb[:, cc, mo * 128 : (mo + 1) * 128],
                    fc1T_sb[:, cc, :],
                    start=(cc == 0),
                    stop=False,
                )
            for cc in range(4):
                nc.tensor.matmul(
                    ps[:, 0, :],
                    Wfc_sb[:, 4 + cc, mo * 128 : (mo + 1) * 128],
                    ctx_wT[:, cc, :],
                    start=False,
                    stop=(cc == 3),
                )
            nc.vector.tensor_scalar_add(
                outT_sb[:, mo, :], ps[:, 0, :], btiles["bfcT"][:, mo : mo + 1]
            )
            nc.sync.dma_start(out=outT_d[mo * 128 : (mo + 1) * 128, :], in_=outT_sb[:, mo, :])

        lpool_cm.__exit__(None, None, None)
        smpool_cm.__exit__(None, None, None)
        ewpool_cm.__exit__(None, None, None)
        vpool_cm.__exit__(None, None, None)
        scpool_cm.__exit__(None, None, None)
        ppool_cm.__exit__(None, None, None)

    ns = _split_multi_waits(nc)
    print(f"[kernel] split {ns} extra sem waits onto NOPs", file=sys.stderr)
    return nc


_NC_CACHE = None


def _get_nc():
    global _NC_CACHE
    if _NC_CACHE is None:
        _NC_CACHE = build_program()
    return _NC_CACHE


def make_in_maps(inputs):
    f = lambda x: np.ascontiguousarray(np.asarray(x, dtype=np.float32))
    q, k_w, v_w, k_s, v_s = (f(inputs[n]) for n in ["q", "k_w", "v_w", "k_s", "v_s"])
    W = {n: f(inputs[n]) for n in inputs if n.startswith(("W_", "b_"))}

    def bT(v, scale=1.0):
        return np.ascontiguousarray((v * scale).reshape(4, 128).T)

    bf = ml_dtypes.bfloat16
    shared = {
        "Wqs": W["W_qs"], "Wks": W["W_ks"], "Wvs": W["W_vs"],
        "Wqw": W["W_qw"], "Wkw": W["W_kw"].astype(bf), "Wvw": W["W_vw"].astype(bf),
        "Wfc1": W["W_fc1"], "Wfc": W["W_fc"],
        "bqsT": bT(W["b_qs"]), "bksT": bT(W["b_ks"]), "bqwT": bT(W["b_qw"]),
        "bkwT": bT(W["b_kw"]), "bvsT": bT(W["b_vs"]),
        "bfc1T": bT(W["b_fc1"], 0.5), "bfcT": bT(W["b_fc"], 0.5),
        "bvw": W["b_vw"],
    }
    in_maps = []
    for c in range(N_CORES):
        b, half = divmod(c, 2)
        blk = slice(half * NBH, half * NBH + NBH)
        ks_r = np.roll(k_s[b], -half * NBH, axis=0)
        vs_r = np.roll(v_s[b], -half * NBH, axis=0)
        m = dict(shared)
        m["qT"] = np.ascontiguousarray(q[b].T)
        m["kwT"] = np.ascontiguousarray(k_w[b, blk].reshape(NTOK, D).T.astype(bf))
        m["vwT"] = np.ascontiguousarray(v_w[b, blk].reshape(NTOK, D).T.astype(bf))
        m["ksT"] = np.ascontiguousarray(ks_r.T)
        m["vsT"] = np.ascontiguousarray(vs_r.T)
        in_maps.append(m)
    return in_maps


def run_cores(inputs, trace=False):
    nc = _get_nc()
    in_maps = make_in_maps(inputs)
    res = run_bass_kernel_spmd(nc, in_maps, list(range(N_CORES)), trace=trace)
    return res


def assemble(res):
    out = np.empty((B, LQ, D), dtype=np.float32)
    for b in range(B):
        out[b] = (res.results[2 * b]["outT"] + res.results[2 * b + 1]["outT"]).T
    return out


def kernel(**inputs) -> np.ndarray:
    res = run_cores(inputs, trace=False)
    return assemble(res)


if __name__ == "__main__":
    import reference

    inp = {k: np.asarray(v) for k, v in reference.setup_inputs().items()}
    out = kernel(**inp)
    exp = np.asarray(reference.reference(**inp))
    err = np.abs(out - exp).max() / np.abs(exp).max()
    print("max rel err:", err)
